# revision 16
# baseline (speedup 1.0000x reference)
"""BEVPoolV2 (segment_reduce) Trainium2 kernel.

Computation: out[rb[p]] += depth.flat[rd[p]] * feat2d[rf[p]]  for p < n_points,
out shape [40000, 80] -> (1, 1, 200, 200, 80).

Strategy (8 NeuronCores, SPMD, no collectives):
  - Host sorts points by BEV bin; bins are sharded contiguously across the 8
    cores (5000 bins each), so each core produces a disjoint slice of the
    output and results are concatenated on the host.
  - Each core's bins form windows of W=50 bins. A window's points are padded
    to a multiple of 128 and processed as 128-point "chunks" (M chunks per
    window, M equalized across windows so all cores run one static program).
  - Feature rows are gathered on-device with the GPSIMD dma_gather firmware
    (mlp ucode library). dma_gather takes int16 row indices, so each core's
    windows are split into 4 "quarters" and the host builds a per-quarter
    compacted feature table (unique rows used by that quarter, < 32768 by
    construction) with rows padded to 512B.
  - Depth weights (one f32 per point) are host-gathered and streamed in (the
    device has no efficient 4-byte-granularity gather).
  - Per chunk: the vector engine builds S[p, i] = (bin_local[p] == i) *
    depth[p]; the PE accumulates psum[W, C] += S^T @ feat_chunk over the
    window's chunks; the scalar engine evacuates PSUM and the sync engine
    DMAs window rows to the per-core output slice.
  - Raw Bass (Bacc) with explicit semaphores: this toolchain rejects inline
    multi-waits, so every wait is a standalone wait_ge instruction.
"""

import numpy as np

import concourse.bacc as bacc
import concourse.bass as bass
import concourse.mybir as mybir
from concourse.bass_utils import run_bass_kernel_spmd
from concourse.library_config import mlp

# Problem constants (hardcoded per contest contract)
P = 128              # points per chunk == PE contraction dim
C = 80               # feature channels
CPAD = 128           # padded row length (512B) for dma_gather
N_CORES = 8
N_BINS = 40000       # B * oD * oH * oW
BINS_PER_CORE = N_BINS // N_CORES   # 5000
W = 50               # bins per window
NW = BINS_PER_CORE // W             # windows per core (100)
NQ = 4               # quarters per core (one compact feat table each)
N_FEAT = 67584       # B * N * iH * iW feature-table rows
TQ = 32768           # compact table rows (int16-indexable)

GROUP = 8            # max chunks per gather group (dma_gather tops out at 1024 idxs)
FB = 6               # feat/S buffer ring depth (groups in flight)
PSB = 2              # psum buffers (windows in flight on PE)
EVB = 4              # evacuation buffers (windows in flight to HBM)


def _plan_groups(M, nw=NW, group=GROUP):
    """Group chunks for gathers; groups never span quarter boundaries."""
    NCH = nw * M
    qch = NCH // NQ
    groups = []  # (start_chunk, size, quarter)
    for q in range(NQ):
        s = q * qch
        while s < (q + 1) * qch:
            sz = min(group, (q + 1) * qch - s)
            groups.append((s, sz, q))
            s += sz
    return NCH, groups


def build_kernel(M, nw=NW, w=W, c=C, cpad=CPAD, tq=TQ, group=GROUP):
    """Raw-Bacc single-core module; all cores run it SPMD with different data."""
    NCH, groups = _plan_groups(M, nw, group)
    NG = len(groups)
    chunk_group = {}
    for gi, (s, sz, q) in enumerate(groups):
        for j in range(sz):
            chunk_group[s + j] = (gi, j)
    gend = [g[0] + g[1] for g in groups]   # chunks completed after group gi

    nc = bacc.Bacc("TRN2", dynamic_dma_scratch_size=32768)
    rf16 = nc.declare_dram_parameter("rf16", [P, NCH * 8], mybir.dt.int16, isOutput=False)
    dv = nc.declare_dram_parameter("dv", [P, NCH], mybir.dt.float32, isOutput=False)
    rbl = nc.declare_dram_parameter("rbl", [P, NCH], mybir.dt.float32, isOutput=False)
    iota = nc.declare_dram_parameter("iota", [P, w], mybir.dt.float32, isOutput=False)
    tabs = [
        nc.declare_dram_parameter(f"tab{q}", [tq, cpad], mybir.dt.float32, isOutput=False)
        for q in range(NQ)
    ]
    bev_out = nc.declare_dram_parameter("bev_out", [nw * w, c], mybir.dt.float32, isOutput=True)

    from contextlib import ExitStack
    with ExitStack() as ctx:
        rf_t = ctx.enter_context(nc.sbuf_tensor("rf_t", [P, NCH * 8], mybir.dt.int16))
        dv_t = ctx.enter_context(nc.sbuf_tensor("dv_t", [P, NCH], mybir.dt.float32))
        rbl_t = ctx.enter_context(nc.sbuf_tensor("rbl_t", [P, NCH], mybir.dt.float32))
        iota_t = ctx.enter_context(nc.sbuf_tensor("iota_t", [P, w], mybir.dt.float32))
        feat_t = ctx.enter_context(nc.sbuf_tensor("feat_t", [P, FB, group, cpad], mybir.dt.float32))
        s_t = ctx.enter_context(nc.sbuf_tensor("s_t", [P, FB, group, w], mybir.dt.float32))
        ev_t = ctx.enter_context(nc.sbuf_tensor("ev_t", [w, EVB, c], mybir.dt.float32))
        ps_ts = [ctx.enter_context(nc.psum_tensor(f"ps{i}_t", [w, c], mybir.dt.float32))
                 for i in range(PSB)]
        load_sem = ctx.enter_context(nc.semaphore("load_sem"))
        gather_sems = [ctx.enter_context(nc.semaphore(f"gather_sem{i}")) for i in range(FB)]
        s_sem = ctx.enter_context(nc.semaphore("s_sem"))
        pe_sem = ctx.enter_context(nc.semaphore("pe_sem"))
        act_sem = ctx.enter_context(nc.semaphore("act_sem"))
        out_sems = [ctx.enter_context(nc.semaphore(f"out_sem{i}")) for i in range(EVB)]
        block = ctx.enter_context(nc.Block())

        @block.sync
        def _(sync):
            sync.dma_start(out=rf_t[:], in_=rf16[:]).then_inc(load_sem, 16)
            sync.dma_start(out=dv_t[:], in_=dv[:]).then_inc(load_sem, 16)
            sync.dma_start(out=rbl_t[:], in_=rbl[:]).then_inc(load_sem, 16)
            sync.dma_start(out=iota_t[:], in_=iota[:]).then_inc(load_sem, 16)
            for wi in range(nw):
                sync.wait_ge(act_sem, wi + 1)
                sync.dma_start(
                    out=bev_out[wi * w:(wi + 1) * w, :], in_=ev_t[:, wi % EVB, :]
                ).then_inc(out_sems[wi % EVB], 16)
            for sl in range(EVB):
                n_dmas = (nw - sl + EVB - 1) // EVB
                sync.wait_ge(out_sems[sl], 16 * n_dmas)

        @block.gpsimd
        def _(gpsimd):
            gpsimd.load_library(mlp)
            gpsimd.wait_ge(load_sem, 64)
            for gi, (s, sz, q) in enumerate(groups):
                if gi >= FB:
                    gpsimd.wait_ge(pe_sem, gend[gi - FB])
                gpsimd.dma_gather(
                    out_ap=feat_t[:, gi % FB, 0:sz, :],
                    in_ap=tabs[q][:],
                    idxs_ap=rf_t[:, s * 8:(s + sz) * 8],
                    num_idxs=sz * P,
                    num_idxs_reg=sz * P,
                    elem_size=cpad,
                ).then_inc(gather_sems[gi % FB], 16)

        @block.vector
        def _(vector):
            vector.wait_ge(load_sem, 64)
            for gi, (s, sz, q) in enumerate(groups):
                if gi >= FB:
                    vector.wait_ge(pe_sem, gend[gi - FB])
                vector.tensor_tensor(
                    out=s_t[:, gi % FB, 0:sz, :],
                    in0=rbl_t[:, s:s + sz].unsqueeze(2).to_broadcast([P, sz, w]),
                    in1=iota_t[:].unsqueeze(1).to_broadcast([P, sz, w]),
                    op=mybir.AluOpType.is_equal,
                ).then_inc(s_sem, 1)
                vector.wait_ge(s_sem, 2 * gi + 1)
                vector.tensor_tensor(
                    out=s_t[:, gi % FB, 0:sz, :],
                    in0=s_t[:, gi % FB, 0:sz, :],
                    in1=dv_t[:, s:s + sz].unsqueeze(2).to_broadcast([P, sz, w]),
                    op=mybir.AluOpType.mult,
                ).then_inc(s_sem, 1)

        @block.tensor
        def _(tensor):
            seen_group = -1
            for ch in range(NCH):
                gi, cidx = chunk_group[ch]
                wi, k = divmod(ch, M)
                if gi != seen_group:
                    tensor.wait_ge(s_sem, 2 * (gi + 1))
                    tensor.wait_ge(gather_sems[gi % FB], 16 * (gi // FB + 1))
                    seen_group = gi
                if k == 0 and wi >= PSB:
                    tensor.wait_ge(act_sem, wi - PSB + 1)
                tensor.matmul(
                    out=ps_ts[wi % PSB][:],
                    lhsT=s_t[:, gi % FB, cidx, :],
                    rhs=feat_t[:, gi % FB, cidx, 0:c],
                    start=(k == 0),
                    stop=(k == M - 1),
                ).then_inc(pe_sem, 1)

        @block.scalar
        def _(scalar):
            for wi in range(nw):
                scalar.wait_ge(pe_sem, (wi + 1) * M)
                if wi >= EVB:
                    scalar.wait_ge(out_sems[wi % EVB], 16 * (wi // EVB))
                scalar.copy(
                    out=ev_t[:, wi % EVB, :],
                    in_=ps_ts[wi % PSB][:],
                ).then_inc(act_sem, 1)

    nc.compile()
    return nc


def _preprocess(ranks_depth, ranks_feat, ranks_bev, n_points, depth_flat, feat2d):
    """Sort points by bin, pack into (core, window, chunk) layout, compact
    per-quarter feature tables, host-gather depth weights."""
    n = int(n_points)
    rd = np.asarray(ranks_depth[:n]).astype(np.int64)
    rf = np.asarray(ranks_feat[:n]).astype(np.int64)
    rb = np.asarray(ranks_bev[:n]).astype(np.int64)

    order = np.argsort(rb, kind="stable")
    rd_s, rf_s, rb_s = rd[order], rf[order], rb[order]

    n_gwin = N_CORES * NW
    win_id = rb_s // W
    counts = np.bincount(win_id, minlength=n_gwin)
    M = max(1, int(-(-counts.max() // P)))
    # quarter boundaries need NCH % NQ == 0 -> NW % NQ == 0 holds (100 % 4)
    NCH = NW * M
    npts = NCH * P

    starts = np.zeros(n_gwin + 1, dtype=np.int64)
    starts[1:] = np.cumsum(counts)
    r = np.arange(n, dtype=np.int64) - starts[win_id]
    core = win_id // NW
    dst = (win_id % NW) * (M * P) + r

    rf_pad = np.zeros((N_CORES, npts), dtype=np.int64)
    rbl_pad = np.zeros((N_CORES, npts), dtype=np.float32)
    dv_pad = np.zeros((N_CORES, npts), dtype=np.float32)
    rf_pad[core, dst] = rf_s
    rbl_pad[core, dst] = (rb_s % W).astype(np.float32)
    dv_pad[core, dst] = depth_flat[rd_s]          # dummies keep dv=0

    # per-(core, quarter) compacted tables + int16 indices
    qpts = npts // NQ
    tabs = np.zeros((N_CORES, NQ, TQ, CPAD), dtype=np.float32)
    rf_c = np.zeros((N_CORES, npts), dtype=np.int16)
    for cc in range(N_CORES):
        for q in range(NQ):
            sl = slice(q * qpts, (q + 1) * qpts)
            uniq, inv = np.unique(rf_pad[cc, sl], return_inverse=True)
            assert len(uniq) <= TQ, f"quarter table overflow: {len(uniq)}"
            tabs[cc, q, :len(uniq), :C] = feat2d[uniq]
            rf_c[cc, sl] = inv.astype(np.int16)

    # int16 index wrap: value for (chunk c, partition p) at
    # [16k + p%16, c*8 + p//16] for k in 0..7
    A = rf_c.reshape(N_CORES, NCH, 8, 16)
    rf16 = np.ascontiguousarray(
        np.tile(A.transpose(0, 3, 1, 2).reshape(N_CORES, 16, NCH * 8), (1, 8, 1))
    )

    def to_pc(a):
        return np.ascontiguousarray(a.reshape(N_CORES, NCH, P).transpose(0, 2, 1))

    return rf16, to_pc(dv_pad), to_pc(rbl_pad), tabs, M


def make_in_maps(inputs):
    depth_flat = np.asarray(inputs["depth"], dtype=np.float32).ravel()
    feat2d = np.ascontiguousarray(
        np.asarray(inputs["feat"], dtype=np.float32).reshape(N_FEAT, C))
    rf16, dv_pc, rbl_pc, tabs, M = _preprocess(
        inputs["ranks_depth"], inputs["ranks_feat"], inputs["ranks_bev"],
        inputs["n_points"], depth_flat, feat2d,
    )
    iota_v = np.broadcast_to(np.arange(W, dtype=np.float32), (P, W)).copy()
    in_maps = []
    for cc in range(N_CORES):
        m = {
            "rf16": rf16[cc],
            "dv": dv_pc[cc],
            "rbl": rbl_pc[cc],
            "iota": iota_v,
        }
        for q in range(NQ):
            m[f"tab{q}"] = tabs[cc, q]
        in_maps.append(m)
    return in_maps, M


def kernel(ranks_depth, ranks_feat, ranks_bev, n_points, depth, feat):
    in_maps, M = make_in_maps(dict(
        ranks_depth=ranks_depth, ranks_feat=ranks_feat, ranks_bev=ranks_bev,
        n_points=n_points, depth=depth, feat=feat,
    ))
    nc = build_kernel(M)
    res = run_bass_kernel_spmd(nc, in_maps, list(range(N_CORES)))
    out = np.concatenate([res.results[cc]["bev_out"] for cc in range(N_CORES)], axis=0)
    return out.reshape(1, 1, 200, 200, C)
